# revision 1
# baseline (speedup 1.0000x reference)
"""Trainium2 Bass kernel for nn_ConvAttentionBlock (B=8, H=W=64, C=HC=128).

Sharding: data-parallel over batch — each of the 8 NeuronCores runs the full
attention block for one [64*64, 128] image.

Architecture (per core, NT=4096 tokens): the Activation engine's exp over
the 4096x4096 score matrix is ~109us busy minimum (1 elem/cycle/partition
@1.2GHz) — the kernel keeps ACT as pure an exp stream as possible and
hides everything else under it:

  - ONE shared PSUM ring ("s" tag, 3 bufs x 2 banks) serves QKV scratch,
    score tiles, l-transpose batches and proj tiles, so there is no
    pool-transition serialization between phases; +2 banks hold the
    attT accumulator.
  - phase A emits x DMAs first (weights ride ACT/Pool DGE queues), spins
    a short PE p-state warmup, computes transposes + k GEMMs + q0/q1;
    remaining q GEMMs, all v GEMMs, and k4-7 copyouts are drip-fed one
    step per kt into stripe 0's loop.
  - q copyouts ride DVE (tensor_scalar add), k copyouts ACT.
  - phase C: per kt: S^T = kT_kt^T qT (2 matmuls) -> ONE 1024-wide exp
    (bf16 out) -> attT += v_kt^T E (2 matmuls, PSUM accum). Emission is
    software-pipelined (S(kt+1) before att(kt)) for the in-order PE queue.
  - l partials: bf16 pair-sum T = E(2t)+E(2t+1) on DVE (2x packed mode),
    then acc(f32) += T split DVE cols [0,512) / Pool [512,1024).
  - phase D per stripe, one step per kt of the next stripe: attT copyout
    halves at the boundary, batched PE transposes of acc + DVE reduce +
    reciprocal per half, proj attT^T wp (all-bf16), stt on DVE, relu on
    Pool, DMA out per half.

S/QKV GEMMs float32r (TF32); E/v/attT/wp path bf16. Total rel-err ~1e-2
budget vs the 2e-2 gate.
"""

import numpy as np

try:
    import concourse.bass as bass
except ImportError:  # pragma: no cover - fallback for bare containers
    import sys
    for p in ("/opt/trn_rl_repo", "/root/.axon_site/_ro/trn_rl_repo"):
        if p not in sys.path:
            sys.path.insert(0, p)
    import concourse.bass as bass

import concourse.mybir as mybir
import concourse.tile as tile
from concourse import bacc
from concourse.bass import ts
from concourse.bass_utils import run_bass_kernel_spmd
from concourse.masks import make_identity

F32 = mybir.dt.float32
F32R = mybir.dt.float32r
BF16 = mybir.dt.bfloat16
FP16 = mybir.dt.float16
AF = mybir.ActivationFunctionType
OP = mybir.AluOpType

B, H, W = 8, 64, 64
NT = H * W            # 4096 tokens per image
C = HC = 128
P = 128
NCH = NT // P         # 32 128-token chunks
NG = NT // 512        # 8 512-token groups
CSHIFT = 50.0         # exp(s - CSHIFT): scores empirically within [-84, 94]
STRIPE = 1024
JC = STRIPE // 512    # 512-col matmuls per S tile
NS = NT // STRIPE     # 4 stripes
SCH = STRIPE // P     # 128-token chunks per stripe (8)
L2D = 512             # acc += T split: DVE cols [0,512), Pool [512,1024)


def f(ap):
    return ap.bitcast(F32)


def build(NT=NT, reps=1, hw_loop=None, bf16_v=False, warm=12,
          bf16_e=True, exp_w=1024, bf16_s=False):
    VDT = BF16 if bf16_v else F32R
    EDT = BF16 if bf16_e else F32R   # dtype of E/v/attT/wp (must match)
    SDT = FP16 if bf16_s else F32R   # dtype of qT/kT (S matmul operands)

    nc = bacc.Bacc(None, target_bir_lowering=False)
    x_d = nc.dram_tensor("x", [NT, C], F32, kind="ExternalInput")
    wq_d = nc.dram_tensor("wq", [C, HC], F32R, kind="ExternalInput")
    wk_d = nc.dram_tensor("wk", [C, HC], F32R, kind="ExternalInput")
    wv_d = nc.dram_tensor("wv", [C, HC], VDT, kind="ExternalInput")
    wp_d = nc.dram_tensor("wp", [HC, C], EDT, kind="ExternalInput")
    bq_d = nc.dram_tensor("bq", [HC, 1], F32, kind="ExternalInput")
    bk_d = nc.dram_tensor("bk", [HC, 1], F32, kind="ExternalInput")
    cv_d = nc.dram_tensor("cvec", [1, C], F32, kind="ExternalInput")
    out_d = nc.dram_tensor("out", [NT, C], F32, kind="ExternalOutput")

    x_src = x_d[:].rearrange("(n p) c -> p n c", p=P)    # [128, NCH, C]
    out_dst = out_d[:].rearrange("(n p) c -> p n c", p=P)

    with tile.TileContext(nc) as tc:
        with (
            tc.tile_pool(name="consts", bufs=1) as consts,
            tc.tile_pool(name="big", bufs=1) as big,
        ):
            # ---- constants ----
            wq_t = consts.tile([C, HC], F32R)
            wk_t = consts.tile([C, HC], F32R)
            wv_t = consts.tile([C, HC], VDT)
            wp_t = consts.tile([HC, C], EDT)
            bq_t = consts.tile([HC, 1], F32)
            bk_t = consts.tile([HC, 1], F32)
            ident = consts.tile([P, P], F32)
            cvec_bc = consts.tile([P, C], F32)
            nshift = consts.tile([P, 1], F32)
            nc.vector.memset(nshift[:], -CSHIFT)
            # ident first: the Pool queue must reach affine_select before
            # its (software-DGE) weight DMAs, or the PE warmup stalls ~3us.
            # Weights ride the ACT/Pool DGE queues; SP is reserved for x.
            make_identity(nc, ident[:])
            nc.scalar.dma_start(bk_t[:], bk_d[:])
            nc.scalar.dma_start(bq_t[:], bq_d[:])
            nc.gpsimd.dma_start(wk_t[:], wk_d[:])
            nc.gpsimd.dma_start(wq_t[:], wq_d[:])
            nc.gpsimd.dma_start(wv_t[:], wv_d[:])
            nc.gpsimd.dma_start(wp_t[:], wp_d[:])
            nc.gpsimd.dma_start(cvec_bc[:], cv_d[:].to_broadcast([P, C]))

            # ---- persistent tiles ----
            x_nat = big.tile([P, NCH, C], F32)     # x + cvec (residual input)
            xT = big.tile([C, NCH, P], F32R)       # x transposed [C, token]
            qT = big.tile([HC, NT], SDT)
            kT = big.tile([HC, NT], SDT)
            v = big.tile([P, NCH, HC], EDT)        # [token, HC]
            acc = big.tile([P, NT], F32)           # exp partials (col-split)
            l32 = big.tile([P, NCH], F32)
            recip_l = big.tile([P, NCH], F32)

            def emit(rep):
                with (
                    tc.tile_pool(name="ps", bufs=1, space="PSUM") as ps,
                    tc.tile_pool(name="epool", bufs=8) as epool,
                    tc.tile_pool(name="tpool", bufs=3) as tpool,
                    tc.tile_pool(name="attp", bufs=2) as attp,
                    tc.tile_pool(name="respool", bufs=3) as respool,
                ):
                    def s_tile():
                        return ps.tile([P, STRIPE], F32, tag="s", bufs=3, name="sbuf")

                    def att_tile():
                        return ps.tile([HC, STRIPE], F32, tag="att", bufs=1, name="attbuf")

                    # ---- Phase A ----
                    for g in range(NG):
                        nc.sync.dma_start(
                            x_nat[:, ts(g, 4), :], x_src[:, ts(g, 4), :]
                        )
                    att_ps = att_tile()   # stripe 0 accumulator, warm scratch
                    for w in range(warm):
                        nc.tensor.transpose(
                            att_ps[:, (w % 4) * P:(w % 4 + 1) * P],
                            ident[:], ident[:])

                    def emit_transposes(g):
                        tp = s_tile()
                        for u in range(4):
                            i = g * 4 + u
                            nc.tensor.transpose(
                                tp[:, u * P:(u + 1) * P],
                                x_nat[:, i, :], ident[:])
                        nc.vector.tensor_copy(
                            xT[:, ts(g, 4), :],
                            tp[:, 0:512].rearrange("p (u q) -> p u q", u=4))

                    def emit_k(g, kq):
                        nc.tensor.matmul(kq[:, 512:1024], wk_t[:],
                                         xT[:, ts(g, 4), :],
                                         start=True, stop=True)

                    def emit_kcopy(g, kq):
                        nc.scalar.activation(kT[:, ts(g, 512)],
                                             kq[:, 512:1024],
                                             AF.Identity, bias=bk_t[:, 0:1],
                                             scale=1.0)

                    def emit_q(g, kq):
                        nc.tensor.matmul(kq[:, 0:512], wq_t[:],
                                         xT[:, ts(g, 4), :],
                                         start=True, stop=True)

                    def emit_qcopy(g, kq):
                        nc.vector.tensor_scalar_add(
                            qT[:, ts(g, 512)], kq[:, 0:512], bq_t[:, 0:1])

                    def emit_cvec(g):
                        for u in range(4):
                            i = g * 4 + u
                            nc.gpsimd.tensor_tensor(
                                x_nat[:, i, :], x_nat[:, i, :], cvec_bc[:],
                                OP.add)

                    def emit_v(g, half=None):
                        # f32r narrow-output matmuls run at 4 cyc/row, so a
                        # v group is ~853ns of PE; callers can split halves.
                        vp = s_tile()
                        rng = range(4) if half is None else range(2 * half, 2 * half + 2)
                        for u in rng:
                            i = g * 4 + u
                            nc.tensor.matmul(
                                vp[:, u * P:(u + 1) * P], xT[:, i, :],
                                wv_t[:], start=True, stop=True)
                        nc.vector.tensor_copy(
                            v[:, ts(g, 4), :],
                            vp[:, 0:512].rearrange("p (u q) -> p u q", u=4))

                    # phase A proper — only stripe 0 kt=0's critical path:
                    # transposes of groups 0-1, k0/q0/k1/q1 (+copyouts), v0.
                    # Everything else (t2-7, k2-7, v1-7, q2-7) drips one step
                    # per kt into stripe 0's loop, deadline-ordered.
                    emit_transposes(0)
                    emit_transposes(1)
                    for g in range(2):
                        kq = s_tile()
                        emit_k(g, kq)
                        emit_q(g, kq)
                        emit_kcopy(g, kq)
                        emit_qcopy(g, kq)
                    emit_v(0)
                    emit_cvec(0)
                    emit_cvec(1)

                    def a_steps():
                        """Leftover QKV work, one step per kt of stripe 0.
                        v(g) needs t(g), used from att(4g); k(g) used from
                        S(4g); q2-7 only by stripes 1-3."""
                        emit_v(1)
                        yield
                        for g in range(2, NG):      # t(g), k(g), v(g) chain
                            emit_transposes(g)
                            emit_cvec(g)
                            yield
                            kq = s_tile()
                            emit_k(g, kq)
                            emit_kcopy(g, kq)
                            yield
                            vp = s_tile()
                            for u in range(2):
                                i = g * 4 + u
                                nc.tensor.matmul(
                                    vp[:, u * P:(u + 1) * P], xT[:, i, :],
                                    wv_t[:], start=True, stop=True)
                            yield
                            for u in range(2, 4):
                                i = g * 4 + u
                                nc.tensor.matmul(
                                    vp[:, u * P:(u + 1) * P], xT[:, i, :],
                                    wv_t[:], start=True, stop=True)
                            nc.vector.tensor_copy(
                                v[:, ts(g, 4), :],
                                vp[:, 0:512].rearrange("p (u q) -> p u q", u=4))
                            yield
                        for g in range(2, NG):      # q2-7
                            kq = s_tile()
                            emit_q(g, kq)
                            emit_qcopy(g, kq)
                            yield

                    # ---- Phase C + interleaved phase D ----
                    def emit_S(s, kt):
                        sp = s_tile()
                        for jc in range(JC):
                            nc.tensor.matmul(
                                sp[:, ts(jc, 512)], kT[:, ts(kt, P)],
                                qT[:, s * STRIPE + jc * 512:
                                      s * STRIPE + (jc + 1) * 512],
                                start=True, stop=True)
                        return sp

                    def emit_exp(sp):
                        E = epool.tile([P, STRIPE], EDT, tag="e")
                        for x0 in range(0, STRIPE, exp_w):
                            nc.scalar.activation(E[:, x0:x0 + exp_w],
                                                 sp[:, x0:x0 + exp_w], AF.Exp,
                                                 bias=nshift[:, 0:1], scale=1.0)
                        return E

                    def emit_att(att_ps, E, kt):
                        for jc in range(JC):
                            nc.tensor.matmul(
                                att_ps[:, ts(jc, 512)], v[:, kt, :],
                                E[:, ts(jc, 512)],
                                start=(kt == 0), stop=(kt == NCH - 1))

                    def ef(ap):
                        return ap if bf16_e else f(ap)

                    def emit_lpair(s, E0, E1, pair):
                        q0 = s * STRIPE
                        T = tpool.tile([P, STRIPE], EDT, tag="t")
                        nc.vector.tensor_tensor(ef(T[:]), ef(E0[:]), ef(E1[:]),
                                                OP.add)
                        if pair == 0:
                            nc.vector.tensor_copy(acc[:, q0:q0 + L2D],
                                                  ef(T[:, 0:L2D]))
                            nc.gpsimd.tensor_copy(
                                acc[:, q0 + L2D:q0 + STRIPE],
                                ef(T[:, L2D:STRIPE]))
                        else:
                            nc.vector.tensor_tensor(
                                acc[:, q0:q0 + L2D], acc[:, q0:q0 + L2D],
                                ef(T[:, 0:L2D]), OP.add)
                            nc.gpsimd.tensor_tensor(
                                acc[:, q0 + L2D:q0 + STRIPE],
                                acc[:, q0 + L2D:q0 + STRIPE],
                                ef(T[:, L2D:STRIPE]), OP.add)

                    def phase_d_steps(s, attT_t):
                        """Phase-D work for stripe s, one instruction per
                        step, consumed during the next stripe's kt loop."""
                        res = respool.tile([P, SCH, C], F32, tag="res")
                        for half in range(2):
                            tq = s_tile()
                            for u in range(4):
                                i = s * SCH + half * 4 + u
                                nc.tensor.transpose(
                                    tq[:, u * P:(u + 1) * P],
                                    acc[:, ts(i, P)], ident[:])
                                yield
                            h0 = s * SCH + half * 4
                            nc.vector.tensor_reduce(
                                l32[:, h0:h0 + 4],
                                tq[:, 0:512].rearrange("p (u q) -> p u q", u=4),
                                axis=mybir.AxisListType.X, op=OP.add)
                            yield
                            nc.vector.reciprocal(
                                recip_l[:, h0:h0 + 4], l32[:, h0:h0 + 4])
                            yield
                        for half in range(2):
                            pq = s_tile()
                            for u in range(4):
                                c8 = half * 4 + u
                                nc.tensor.matmul(pq[:, u * P:(u + 1) * P],
                                                 attT_t[:, ts(c8, P)],
                                                 wp_t[:], start=True, stop=True)
                                yield
                            for u in range(4):
                                c8 = half * 4 + u
                                i = s * SCH + c8
                                nc.vector.scalar_tensor_tensor(
                                    res[:, c8, :], pq[:, u * P:(u + 1) * P],
                                    recip_l[:, i:i + 1],
                                    x_nat[:, i, :], op0=OP.mult, op1=OP.add)
                                yield
                            nc.gpsimd.tensor_scalar_max(
                                res[:, ts(half, 4), :], res[:, ts(half, 4), :],
                                0.0)
                            nc.sync.dma_start(
                                out_dst[:, s * SCH + half * 4:
                                        s * SCH + half * 4 + 4, :],
                                res[:, ts(half, 4), :])
                            yield

                    pending = a_steps()
                    for s in range(NS):
                        if s > 0:
                            att_ps = att_tile()
                        sp = emit_S(s, 0)
                        E_prev = None
                        for kt in range(NCH):
                            E = emit_exp(sp)
                            if kt < NCH - 1:
                                sp = emit_S(s, kt + 1)
                            emit_att(att_ps, E, kt)
                            if kt % 2 == 1:
                                emit_lpair(s, E_prev, E, kt // 2)
                            E_prev = E
                            if pending is not None and (s == 0 or kt >= 2):
                                next(pending, None)
                        if pending is not None:
                            for _ in pending:
                                pass
                        # attT copyout must be emitted BEFORE the next
                        # stripe's att matmuls touch the (bufs=1) psum tile,
                        # so Tile's program-order dep tracking serializes
                        # copy -> overwrite and not the reverse.
                        attT_t = attp.tile([HC, STRIPE], EDT, tag="att")
                        nc.vector.tensor_copy(attT_t[:, 0:512],
                                              att_ps[:, 0:512])
                        if s == NS - 1:
                            # tail: ACT is idle after the last exp; shorten
                            # the DVE-serial drain chain
                            nc.scalar.activation(attT_t[:, 512:1024],
                                                 att_ps[:, 512:1024],
                                                 AF.Copy, bias=0.0, scale=1.0)
                        else:
                            nc.vector.tensor_copy(attT_t[:, 512:1024],
                                                  att_ps[:, 512:1024])
                        pending = phase_d_steps(s, attT_t)
                    for _ in pending:
                        pass

            if hw_loop is not None:
                with tc.For_i(0, hw_loop) as _i:
                    emit(0)
            else:
                for _rep in range(reps):
                    emit(_rep)

    nc.finalize()
    return nc


_cached_nc = None


def _make_in_maps(x, wq, bq, wk, bk, wv, bv, wp, bp, bf16_v=False,
                  bf16_e=True):
    cvec = (bv.astype(np.float64) @ wp.astype(np.float64)
            + bp.astype(np.float64)).astype(np.float32).reshape(1, C)
    vdt = mybir.dt.np(BF16) if bf16_v else np.float32
    pdt = mybir.dt.np(BF16) if bf16_e else np.float32
    in_maps = []
    for b in range(B):
        in_maps.append({
            "x": np.ascontiguousarray(x[b].reshape(NT, C), dtype=np.float32),
            "wq": np.ascontiguousarray(wq, dtype=np.float32),
            "wk": np.ascontiguousarray(wk, dtype=np.float32),
            "wv": np.ascontiguousarray(wv).astype(vdt),
            "wp": np.ascontiguousarray(wp).astype(pdt),
            "bq": np.ascontiguousarray(bq.reshape(HC, 1), dtype=np.float32),
            "bk": np.ascontiguousarray(bk.reshape(HC, 1), dtype=np.float32),
            "cvec": cvec,
        })
    return in_maps


def kernel(x, wq, bq, wk, bk, wv, bv, wp, bp):
    global _cached_nc
    x = np.asarray(x)
    if _cached_nc is None:
        _cached_nc = build()
    in_maps = _make_in_maps(np.asarray(x), np.asarray(wq), np.asarray(bq),
                            np.asarray(wk), np.asarray(bk), np.asarray(wv),
                            np.asarray(bv), np.asarray(wp), np.asarray(bp))
    res = run_bass_kernel_spmd(_cached_nc, in_maps, core_ids=list(range(B)))
    out = np.stack([res.results[b]["out"].reshape(H, W, C) for b in range(B)])
    return out.astype(np.float32)

